# revision 17
# baseline (speedup 1.0000x reference)
"""8-core data-parallel fused attention kernel for TRN2 (Bass/Tile), v2.

Problem: B=8, N=1024 (32x32 grid), DIM=1024, 16 heads x 64, axial RoPE on
first 32 channels of each head, softmax attention, output projection.

Sharding: pure data-parallel -- core b computes batch element b end-to-end.

v2 changes over the 328us baseline (all aimed at PE slot count + DMA):
- Merged head layout: each head's 64 dims (32 rot + 32 pass) contiguous on
  64 partitions; head pairs occupy partitions 0-63 / 64-127 of a pair chunk.
  Scores contract d in ONE K=64 matmul per head, and the two heads of a pair
  run CONCURRENTLY as 2x row-tiled (64x128) PE tiles -> half the score slots.
- rotate_half via a PE permute matmul (P^T stationary) on the raw QKV output
  instead of doubling QKV weight columns -> QKV matmuls drop 384 -> 256+32.
- Softmax denominators: V keeps a ones column (row 64 of attn@V output);
  normalization broadcasts 1/den across partitions with a K=2 matmul
  (ones block-pattern stationary) instead of a DRAM-bounce DMA.
- RoPE epilogue split across engines: PSUM readers (cast, tilde*sin) on DVE,
  SBUF-only ops (raw*cos, final add) on idle GPSIMD.
- Input DMAs consolidated into ~13 multi-dim transfers (was 75) so the sync
  engine stops serializing 605ns triggers; priority order feeds the PE fast.
"""

import os
import sys

for _p in ("/opt/trn_rl_repo",):
    if os.path.isdir(_p) and _p not in sys.path:
        sys.path.insert(0, _p)

import numpy as np
import ml_dtypes

import concourse.bass as bass
import concourse.bacc as bacc
import concourse.mybir as mybir
import concourse.tile as tile
from concourse.bass_utils import run_bass_kernel_spmd

P = 128
NTOK = 1024
DIM = 1024
HEADS = 16
HD = 64
ROT = 32
QT = 512          # free-dim tile for matmuls (one PSUM bank of f32)
NQ = NTOK // QT   # 2
BF = mybir.dt.bfloat16
F32 = mybir.dt.float32
AL = mybir.AluOpType
AF = mybir.ActivationFunctionType

LAST_RESULT = None
_BUILT = None


# ---------------------------------------------------------------- host prep

def _axial_tables():
    """cos/sin[t, d] for t=0..1023 (t=h*32+w), d=0..31, exactly as reference."""
    rot_half = 8
    base = np.linspace(1.0, 512.0, rot_half) * np.pi          # (8,)
    th = np.linspace(-1.0, 1.0, 32)[:, None] * base[None, :]  # (32, 8)
    fh = np.repeat(th, 2, axis=-1)                            # (32, 16)
    freqs = np.zeros((32, 32, ROT))
    freqs[:, :, :16] = fh[:, None, :]                         # H-axis channels
    freqs[:, :, 16:] = fh[None, :, :]                         # W-axis channels
    f = freqs.reshape(NTOK, ROT)
    return np.cos(f).astype(np.float32), np.sin(f).astype(np.float32)


def _prep_weights(Wqkv, Wproj, bproj):
    bf = ml_dtypes.bfloat16
    Wq, Wk, Wv = Wqkv[0:DIM], Wqkv[DIM:2 * DIM], Wqkv[2 * DIM:3 * DIM]
    # wqk columns per quad m: [Q pair 2m | Q pair 2m+1 | K pair 2m | K pair 2m+1]
    # each a contiguous 128-row slice of Wq/Wk (head h rows h*64..h*64+63,
    # rot channels first within the head -- already the merged layout).
    blocks = []
    for m in range(4):
        blocks += [Wq[256 * m:256 * m + 128], Wq[256 * m + 128:256 * m + 256],
                   Wk[256 * m:256 * m + 128], Wk[256 * m + 128:256 * m + 256]]
    wqk = np.concatenate(blocks, axis=0)                       # (2048, 1024)

    cos_td, sin_td = _axial_tables()                           # (1024, 32)
    cosP = np.ones((P, NTOK), np.float32)
    sinP = np.zeros((P, NTOK), np.float32)
    cosP[0:32] = cos_td.T
    cosP[64:96] = cos_td.T
    sinP[0:32] = sin_td.T
    sinP[64:96] = sin_td.T

    # rotate_half permutation on the merged pair layout (zero on pass rows):
    # ptl = M @ qraw; matmul computes lhsT.T @ rhs so lhsT = M.T
    M = np.zeros((P, P), np.float32)
    for hh in range(2):
        b = hh * 64
        for i in range(16):
            M[b + 2 * i, b + 2 * i + 1] = -1.0
            M[b + 2 * i + 1, b + 2 * i] = 1.0
    # broadcast patterns for the two K=1 normalize matmuls (custom DVE ops
    # and small matmul operands must sit at partition base 0, so both dens
    # live side-by-side on one partition and the two patterns share row 0):
    # cols 0:128 -> rows 0-63 get rr[0:512]; cols 128:256 -> rows 64-127
    bcw = np.zeros((1, 2 * P), np.float32)
    bcw[0, 0:64] = 1.0
    bcw[0, 128 + 64:128 + 128] = 1.0

    biasT = bproj.reshape(8, P).T.copy()                       # (128, 8)

    def pmajor(wT, ncol):
        # [1024, ncol] -> [128, 8*ncol]: row p holds chunks c=0..7 contiguous
        return np.ascontiguousarray(
            wT.reshape(8, P, ncol).transpose(1, 0, 2).reshape(P, 8 * ncol))

    return {
        "wqk": pmajor(wqk.T, 2048).astype(bf),                 # (128, 16384)
        "wv": pmajor(np.ascontiguousarray(Wv.T), DIM).astype(bf),
        "wp": pmajor(np.ascontiguousarray(Wproj.T), DIM).astype(bf),
        "cosp": np.ascontiguousarray(cosP).astype(bf),
        "sinp": np.ascontiguousarray(sinP).astype(bf),
        "pt": np.ascontiguousarray(M.T).astype(bf),            # (128, 128)
        "bcw": np.ascontiguousarray(bcw).astype(bf),           # (2, 128)
        "biasT": np.ascontiguousarray(biasT.astype(np.float32)),
    }


# ------------------------------------------------------------- bass builder

def _build():
    nc = bacc.Bacc()
    xT_e = nc.declare_dram_parameter("xT", [P, 8 * NTOK], BF, isOutput=False)
    wqk_e = nc.declare_dram_parameter("wqk", [P, 8 * 2048], BF, isOutput=False)
    wv_e = nc.declare_dram_parameter("wv", [P, 8 * DIM], BF, isOutput=False)
    wp_e = nc.declare_dram_parameter("wp", [P, 8 * DIM], BF, isOutput=False)
    cos_e = nc.declare_dram_parameter("cosp", [P, NTOK], BF, isOutput=False)
    sin_e = nc.declare_dram_parameter("sinp", [P, NTOK], BF, isOutput=False)
    pt_e = nc.declare_dram_parameter("pt", [P, P], BF, isOutput=False)
    bcw_e = nc.declare_dram_parameter("bcw", [1, 2 * P], BF, isOutput=False)
    b_e = nc.declare_dram_parameter("biasT", [P, 8], F32, isOutput=False)
    out_e = nc.declare_dram_parameter("out", [DIM, NTOK], F32, isOutput=True)
    debug = bool(os.environ.get("KDEBUG"))
    if debug:
        dbg_e = nc.declare_dram_parameter("dbg", [P, 33344], BF, isOutput=True)

    with tile.TileContext(nc) as tc:
        with (
            tc.tile_pool(name="persist", bufs=1) as persist,
            tc.tile_pool(name="work", bufs=2) as work,
            tc.tile_pool(name="raws", bufs=5) as raws,
            tc.tile_pool(name="work3", bufs=10) as work3,
            tc.tile_pool(name="ps_sc", bufs=2, space="PSUM") as ps_sc_pool,
            tc.tile_pool(name="ps_av", bufs=2, space="PSUM") as ps_av_pool,
            tc.tile_pool(name="ps_mm", bufs=2, space="PSUM") as ps_mm_pool,
        ):
            xT = persist.tile([P, 8, NTOK], BF)
            wqk = persist.tile([P, 8, 2048], BF)
            wv = persist.tile([P, 8, DIM], BF)
            wp = persist.tile([P, 8, DIM], BF)
            cosp = persist.tile([P, NTOK], BF)
            sinp = persist.tile([P, NTOK], BF)
            ptm = persist.tile([P, P], BF)
            bcw = persist.tile([1, 2 * P], BF)
            biasT = persist.tile([P, 8], F32)
            # rope'd QK, merged layout: partition 64*(h%2) + d, chunk = pair
            qm = persist.tile([P, 8, NTOK], BF)
            km = persist.tile([P, 8, NTOK], BF)
            # V with ones column: [k-token partitions, kc, head, 65]
            vaug = persist.tile([P, 8, HEADS * (HD + 1)], BF)
            # attention out, transposed: partition 64*(h%2)+d, chunk h//2
            outT = persist.tile([P, 8, NTOK], BF)

            vaug_r = vaug.rearrange("p n (h e) -> p n h e", e=HD + 1)

            # ---------------- input DMAs (consolidated, priority order)
            # DRAM params are partition-major ([128, 8*ncol], chunks c
            # contiguous per row) so every transfer has >=1KB runs.
            def load3d(dst, src_e, cols, ncol):
                src = src_e.rearrange("p (c n) -> p c n", c=8)
                src = src[:, :, cols] if cols is not None else src[:, :, :]
                nc.sync.dma_start(out=dst, in_=src)

            def load3d_s(dst, src_e, cols, ncol):
                src = src_e.rearrange("p (c n) -> p c n", c=8)
                src = src[:, :, cols] if cols is not None else src[:, :, :]
                nc.scalar.dma_start(out=dst, in_=src)

            load3d(xT[:, :, 0:QT], xT_e, slice(0, QT), NTOK)
            load3d_s(wqk[:, :, 0:512], wqk_e, slice(0, 512), 2048)
            nc.scalar.dma_start(out=cosp[:], in_=cos_e[:, :])
            nc.scalar.dma_start(out=sinp[:], in_=sin_e[:, :])
            nc.scalar.dma_start(out=ptm[:], in_=pt_e[:, :])
            load3d(xT[:, :, QT:NTOK], xT_e, slice(QT, NTOK), NTOK)
            load3d_s(wv[:, :, :], wv_e, None, DIM)
            nc.scalar.dma_start(out=bcw[:], in_=bcw_e[:, :])
            load3d(wqk[:, :, 512:1024], wqk_e, slice(512, 1024), 2048)
            load3d_s(wqk[:, :, 1024:1536], wqk_e, slice(1024, 1536), 2048)
            load3d(wqk[:, :, 1536:2048], wqk_e, slice(1536, 2048), 2048)
            nc.scalar.dma_start(out=biasT[:], in_=b_e[:, :])
            load3d(wp[:, :, :], wp_e, None, DIM)

            # ---------------- V = x @ Wv^T, natural orientation
            def v_units():
                for tt in range(8):
                    for g in range(2):
                        pt = ps_mm_pool.tile([P, QT], F32, tag="ps_mm")
                        for cc in range(8):
                            nc.tensor.matmul(
                                pt[:],
                                xT[:, cc, tt * P:(tt + 1) * P],
                                wv[:, cc, g * QT:(g + 1) * QT],
                                start=(cc == 0), stop=(cc == 7))
                        nc.vector.tensor_copy(
                            vaug_r[:, tt, g * 8:(g + 1) * 8, 0:HD],
                            pt[:].rearrange("p (h d) -> p h d", d=HD))
                        yield
                    nc.vector.memset(vaug_r[:, tt, :, HD:HD + 1], 1.0)

            # ---------------- QKV^T for one quad (pairs 2m, 2m+1) + RoPE
            def rope_tail(item):
                dst, pp, m, ts_, ca, sb = item
                ptl = ps_mm_pool.tile([P, QT], F32, tag="ps_mm")
                nc.tensor.matmul(ptl[:], ptm[:], sb[:],
                                 start=True, stop=True)
                nc.vector.tensor_add(
                    dst[:, 2 * m + pp, ts_], ca[:], ptl[:])

            def qkv_units(m):
                stash = []
                for t2 in range(NQ):
                    ts_ = slice(t2 * QT, (t2 + 1) * QT)
                    for gi, (dst, pp) in enumerate(
                            ((qm, 0), (qm, 1), (km, 0), (km, 1))):
                        pr = ps_mm_pool.tile([P, QT], F32, tag="ps_mm")
                        col0 = m * 512 + gi * P
                        for cc in range(8):
                            nc.tensor.matmul(
                                pr[:],
                                wqk[:, cc, col0:col0 + P],
                                xT[:, cc, ts_],
                                start=(cc == 0), stop=(cc == 7))
                        # emit permute+rope of the group 2 slots back first
                        if len(stash) >= 2:
                            rope_tail(stash.pop(0))
                        ca = raws.tile([P, QT], BF, tag="ca")
                        sb = raws.tile([P, QT], BF, tag="sb")
                        nc.vector.tensor_tensor(
                            ca[:], pr[:], cosp[:, ts_], op=AL.mult)
                        nc.vector.tensor_tensor(
                            sb[:], pr[:], sinp[:, ts_], op=AL.mult)
                        stash.append((dst, pp, m, ts_, ca, sb))
                        yield
                for item in stash:
                    rope_tail(item)
                    yield

            # ---------------- attention for one pair (2 heads, concurrent)
            carry = [None]   # deferred attn@V burst crossing attn tiles

            def attn_units(p, qt):
                qs = slice(qt * QT, (qt + 1) * QT)
                h0, h1 = 2 * p, 2 * p + 1
                pending = None   # previous slot's deferred attn@V burst
                po0 = ps_av_pool.tile([HD + 1, QT], F32, tag="ps_o")
                po1 = ps_av_pool.tile([HD + 1, QT], F32, tag="ps_o")

                def make_av(aTs, kcs, finish):
                    def emit():
                        for aT, kc in zip(aTs, kcs):
                            nc.tensor.matmul(
                                po0[:], vaug_r[:, kc, h0, :], aT[:, 0:QT],
                                start=(kc == 0), stop=(kc == 7))
                            nc.tensor.matmul(
                                po1[:], vaug_r[:, kc, h1, :], aT[:, QT:2 * QT],
                                start=(kc == 0), stop=(kc == 7))
                        if finish:
                            den2 = work.tile([1, 2 * QT], BF, tag="den2")
                            nc.vector.tensor_copy(
                                den2[0:1, 0:QT], po0[HD:HD + 1, :])
                            nc.vector.tensor_copy(
                                den2[0:1, QT:2 * QT], po1[HD:HD + 1, :])
                            rbp = ps_mm_pool.tile([P, QT], F32, tag="ps_mm")
                            nc.tensor.matmul(rbp[:], bcw[0:1, 0:P],
                                             den2[0:1, 0:QT],
                                             start=True, stop=False)
                            nc.tensor.matmul(rbp[:], bcw[0:1, P:2 * P],
                                             den2[0:1, QT:2 * QT],
                                             start=False, stop=True)
                            rbs = work.tile([P, QT], F32, tag="rbs")
                            nc.vector.reciprocal_approx_fast(rbs[:], rbp[:])
                            nc.vector.tensor_tensor(
                                outT[0:HD, p, qs],
                                po0[0:HD, :], rbs[0:HD, :], op=AL.mult)
                            nc.vector.tensor_tensor(
                                outT[HD:2 * HD, p, qs],
                                po1[0:HD, :], rbs[HD:2 * HD, :], op=AL.mult)
                    return emit

                for g4 in range(2):
                    aTs = []
                    kcs = list(range(4 * g4, 4 * g4 + 4))
                    for kc in kcs:
                        ks = slice(kc * P, (kc + 1) * P)
                        pss = ps_sc_pool.tile([P, 2 * QT], F32, tag="ps_s")
                        nc.tensor.matmul(
                            pss[:, 0:QT],
                            km[0:HD, p, ks], qm[0:HD, p, qs],
                            start=True, stop=True)
                        nc.tensor.matmul(
                            pss[:, QT:2 * QT],
                            km[HD:2 * HD, p, ks], qm[HD:2 * HD, p, qs],
                            start=True, stop=True)
                        aT = work3.tile([P, 2 * QT], BF, tag="aT")
                        nc.scalar.activation(aT[:], pss[:], AF.Exp,
                                             scale=0.125)
                        aTs.append(aT)
                        if kc == kcs[0] and carry[0] is not None:
                            carry[0]()
                            carry[0] = None
                        if kc % 2 == 1:
                            yield
                    if pending is not None:
                        pending()
                        yield
                    pending = make_av(aTs, kcs, finish=(g4 == 1))
                carry[0] = pending
                yield

            def attn_flush():
                if carry[0] is not None:
                    carry[0]()
                    carry[0] = None
                yield

            # ---------------- output projection + bias
            def proj_units(qt):
                for ot in range(8):
                    os_ = slice(ot * P, (ot + 1) * P)
                    qs = slice(qt * QT, (qt + 1) * QT)
                    pt = ps_mm_pool.tile([P, QT], F32, tag="ps_mm")
                    for cc in range(8):
                        nc.tensor.matmul(
                            pt[:], wp[:, cc, os_], outT[:, cc, qs],
                            start=(cc == 0), stop=(cc == 7))
                    ys = work.tile([P, QT], F32, tag="ys")
                    nc.vector.tensor_scalar_add(ys[:], pt[:], biasT[:, ot:ot + 1])
                    nc.sync.dma_start(out=out_e[os_, qs], in_=ys[:])
                    yield

            def run(gen):
                for _ in gen:
                    pass

            def weave(a, b, ra=3, rb=1):
                """Generator: alternate ra units from a with rb units from b."""
                a, b = iter(a), iter(b)
                alive_a = alive_b = True
                while alive_a or alive_b:
                    for _ in range(ra):
                        if alive_a:
                            try:
                                next(a)
                            except StopIteration:
                                alive_a = False
                            else:
                                yield
                    for _ in range(rb):
                        if alive_b:
                            try:
                                next(b)
                            except StopIteration:
                                alive_b = False
                            else:
                                yield

            def chain(*gens):
                for g in gens:
                    for _ in g:
                        yield

            run(weave(qkv_units(0), v_units(), 2, 3))
            run(weave(chain(attn_units(0, 0), attn_units(1, 0)),
                      qkv_units(1), 3, 2))
            run(weave(chain(attn_units(2, 0), attn_units(3, 0)),
                      qkv_units(2), 3, 2))
            run(weave(chain(attn_units(4, 0), attn_units(5, 0)),
                      qkv_units(3), 3, 2))
            run(chain(attn_units(6, 0), attn_units(7, 0)))
            run(weave(chain(*[attn_units(p, 1) for p in range(8)],
                            attn_flush()),
                      proj_units(0), 7, 1))
            run(proj_units(1))
            if debug:
                nc.sync.dma_start(out=dbg_e[:, 0:8192],
                                  in_=qm.rearrange("p c n -> p (c n)"))
                nc.sync.dma_start(out=dbg_e[:, 8192:16384],
                                  in_=km.rearrange("p c n -> p (c n)"))
                nc.sync.dma_start(out=dbg_e[:, 16384:24704],
                                  in_=vaug.rearrange("p c n -> p (c n)"))
                nc.sync.dma_start(out=dbg_e[:, 24704:24704 + 128],
                                  in_=ptm[:, :])
                nc.sync.dma_start(out=dbg_e[:, 24832:24832 + 128],
                                  in_=cosp[:, 0:128])
                nc.sync.dma_start(out=dbg_e[:, 24960:24960 + 128],
                                  in_=sinp[:, 0:128])
                nc.sync.dma_start(out=dbg_e[:, 25152:33344],
                                  in_=outT.rearrange("p c n -> p (c n)"))

    nc.compile()
    return nc


def _get_nc():
    global _BUILT
    if _BUILT is None:
        _BUILT = _build()
    return _BUILT


# ------------------------------------------------- tracing support (axon)

def _ensure_trace_hooks():
    """Register the NTFF profile hook that the bare agent image's antenv
    stub lacks, and neuter the artifact upload (no bucket in-container)."""
    import types
    import concourse.bass_utils as bu

    bu.upload_artifacts = lambda tmpdir: f"local:{tmpdir}"
    try:
        from antenv.axon_hooks import get_axon_ntff_profile_hook  # noqa: F401
        return
    except ImportError:
        pass
    mod = types.ModuleType("antenv.axon_hooks")
    _state = {"hook": None}
    mod.set_axon_ntff_profile_hook = lambda h: _state.__setitem__("hook", h)
    mod.get_axon_ntff_profile_hook = lambda: _state["hook"]
    import antenv
    sys.modules["antenv.axon_hooks"] = mod
    antenv.axon_hooks = mod
    try:
        from trn_agent_boot.trn_boot import _ntff_profile_via_ctypes
        hook = _ntff_profile_via_ctypes("/opt/axon/libaxon_pjrt.so")
        if hook is not None:
            mod.set_axon_ntff_profile_hook(hook)
    except Exception as e:  # pragma: no cover
        print(f"NTFF hook install failed: {e!r}")


# ----------------------------------------------------------------- kernel()

def kernel(x, Wqkv, Wproj, bproj):
    global LAST_RESULT
    x = np.asarray(x, np.float32)
    Wqkv = np.asarray(Wqkv, np.float32)
    Wproj = np.asarray(Wproj, np.float32)
    bproj = np.asarray(bproj, np.float32)
    B = x.shape[0]

    base = _prep_weights(Wqkv, Wproj, bproj)
    bf = ml_dtypes.bfloat16
    def xprep(xb):
        t = np.ascontiguousarray(xb.T).reshape(8, 128, NTOK)
        return np.ascontiguousarray(
            t.transpose(1, 0, 2).reshape(128, 8 * NTOK)).astype(bf)

    in_maps = [dict(base, xT=xprep(x[b])) for b in range(B)]
    nc = _get_nc()
    trace = bool(os.environ.get("KBENCH_TRACE"))
    if trace:
        _ensure_trace_hooks()
    res = run_bass_kernel_spmd(
        nc, in_maps, core_ids=list(range(B)), trace=trace)
    LAST_RESULT = res
    out = np.stack([np.asarray(res.results[b]["out"]).T for b in range(B)])
    return np.ascontiguousarray(out.astype(np.float32))


# revision 18
# speedup vs baseline: 1.1316x; 1.1316x over previous
"""8-core data-parallel fused attention kernel for TRN2 (Bass/Tile), v2.

Problem: B=8, N=1024 (32x32 grid), DIM=1024, 16 heads x 64, axial RoPE on
first 32 channels of each head, softmax attention, output projection.

Sharding: pure data-parallel -- core b computes batch element b end-to-end.

v2 changes over the 328us baseline (all aimed at PE slot count + DMA):
- Merged head layout: each head's 64 dims (32 rot + 32 pass) contiguous on
  64 partitions; head pairs occupy partitions 0-63 / 64-127 of a pair chunk.
  Scores contract d in ONE K=64 matmul per head, and the two heads of a pair
  run CONCURRENTLY as 2x row-tiled (64x128) PE tiles -> half the score slots.
- rotate_half via a PE permute matmul (P^T stationary) on the raw QKV output
  instead of doubling QKV weight columns -> QKV matmuls drop 384 -> 256+32.
- Softmax denominators: V keeps a ones column (row 64 of attn@V output);
  normalization broadcasts 1/den across partitions with a K=2 matmul
  (ones block-pattern stationary) instead of a DRAM-bounce DMA.
- RoPE epilogue split across engines: PSUM readers (cast, tilde*sin) on DVE,
  SBUF-only ops (raw*cos, final add) on idle GPSIMD.
- Input DMAs consolidated into ~13 multi-dim transfers (was 75) so the sync
  engine stops serializing 605ns triggers; priority order feeds the PE fast.
"""

import os
import sys

for _p in ("/opt/trn_rl_repo",):
    if os.path.isdir(_p) and _p not in sys.path:
        sys.path.insert(0, _p)

import numpy as np
import ml_dtypes

import concourse.bass as bass
import concourse.bacc as bacc
import concourse.mybir as mybir
import concourse.tile as tile
from concourse.bass_utils import run_bass_kernel_spmd

P = 128
NTOK = 1024
DIM = 1024
HEADS = 16
HD = 64
ROT = 32
QT = 512          # free-dim tile for matmuls (one PSUM bank of f32)
NQ = NTOK // QT   # 2
BF = mybir.dt.bfloat16
F32 = mybir.dt.float32
AL = mybir.AluOpType
AF = mybir.ActivationFunctionType

LAST_RESULT = None
_BUILT = None


# ---------------------------------------------------------------- host prep

def _axial_tables():
    """cos/sin[t, d] for t=0..1023 (t=h*32+w), d=0..31, exactly as reference."""
    rot_half = 8
    base = np.linspace(1.0, 512.0, rot_half) * np.pi          # (8,)
    th = np.linspace(-1.0, 1.0, 32)[:, None] * base[None, :]  # (32, 8)
    fh = np.repeat(th, 2, axis=-1)                            # (32, 16)
    freqs = np.zeros((32, 32, ROT))
    freqs[:, :, :16] = fh[:, None, :]                         # H-axis channels
    freqs[:, :, 16:] = fh[None, :, :]                         # W-axis channels
    f = freqs.reshape(NTOK, ROT)
    return np.cos(f).astype(np.float32), np.sin(f).astype(np.float32)


def _prep_weights(Wqkv, Wproj, bproj):
    bf = ml_dtypes.bfloat16
    Wq, Wk, Wv = Wqkv[0:DIM], Wqkv[DIM:2 * DIM], Wqkv[2 * DIM:3 * DIM]
    # wqk columns per quad m: [Q pair 2m | Q pair 2m+1 | K pair 2m | K pair 2m+1]
    # each a contiguous 128-row slice of Wq/Wk (head h rows h*64..h*64+63,
    # rot channels first within the head -- already the merged layout).
    blocks = []
    for m in range(4):
        blocks += [Wq[256 * m:256 * m + 128], Wq[256 * m + 128:256 * m + 256],
                   Wk[256 * m:256 * m + 128], Wk[256 * m + 128:256 * m + 256]]
    wqk = np.concatenate(blocks, axis=0)                       # (2048, 1024)

    cos_td, sin_td = _axial_tables()                           # (1024, 32)
    cosP = np.ones((P, NTOK), np.float32)
    sinP = np.zeros((P, NTOK), np.float32)
    cosP[0:32] = cos_td.T
    cosP[64:96] = cos_td.T
    sinP[0:32] = sin_td.T
    sinP[64:96] = sin_td.T

    # rotate_half permutation on the merged pair layout (zero on pass rows):
    # ptl = M @ qraw; matmul computes lhsT.T @ rhs so lhsT = M.T
    M = np.zeros((P, P), np.float32)
    for hh in range(2):
        b = hh * 64
        for i in range(16):
            M[b + 2 * i, b + 2 * i + 1] = -1.0
            M[b + 2 * i + 1, b + 2 * i] = 1.0
    # broadcast patterns for the two K=1 normalize matmuls (custom DVE ops
    # and small matmul operands must sit at partition base 0, so both dens
    # live side-by-side on one partition and the two patterns share row 0):
    # cols 0:128 -> rows 0-63 get rr[0:512]; cols 128:256 -> rows 64-127
    bcw = np.zeros((1, 2 * P), np.float32)
    bcw[0, 0:64] = 1.0
    bcw[0, 128 + 64:128 + 128] = 1.0

    biasT = bproj.reshape(8, P).T.copy()                       # (128, 8)

    def pmajor(wT, ncol):
        # [1024, ncol] -> [128, 8*ncol]: row p holds chunks c=0..7 contiguous
        return np.ascontiguousarray(
            wT.reshape(8, P, ncol).transpose(1, 0, 2).reshape(P, 8 * ncol))

    return {
        "wqk": pmajor(wqk.T, 2048).astype(bf),                 # (128, 16384)
        "wv": pmajor(np.ascontiguousarray(Wv.T), DIM).astype(bf),
        "wp": pmajor(np.ascontiguousarray(Wproj.T), DIM).astype(bf),
        "cosp": np.ascontiguousarray(cosP).astype(bf),
        "sinp": np.ascontiguousarray(sinP).astype(bf),
        "pt": np.ascontiguousarray(M.T).astype(bf),            # (128, 128)
        "bcw": np.ascontiguousarray(bcw).astype(bf),           # (2, 128)
        "biasT": np.ascontiguousarray(biasT.astype(np.float32)),
    }


# ------------------------------------------------------------- bass builder

def _build():
    nc = bacc.Bacc()
    xT_e = nc.declare_dram_parameter("xT", [P, 8 * NTOK], BF, isOutput=False)
    wqk_e = nc.declare_dram_parameter("wqk", [P, 8 * 2048], BF, isOutput=False)
    wv_e = nc.declare_dram_parameter("wv", [P, 8 * DIM], BF, isOutput=False)
    wp_e = nc.declare_dram_parameter("wp", [P, 8 * DIM], BF, isOutput=False)
    cos_e = nc.declare_dram_parameter("cosp", [P, NTOK], BF, isOutput=False)
    sin_e = nc.declare_dram_parameter("sinp", [P, NTOK], BF, isOutput=False)
    pt_e = nc.declare_dram_parameter("pt", [P, P], BF, isOutput=False)
    bcw_e = nc.declare_dram_parameter("bcw", [1, 2 * P], BF, isOutput=False)
    b_e = nc.declare_dram_parameter("biasT", [P, 8], F32, isOutput=False)
    out_e = nc.declare_dram_parameter("out", [DIM, NTOK], F32, isOutput=True)
    debug = bool(os.environ.get("KDEBUG"))
    if debug:
        dbg_e = nc.declare_dram_parameter("dbg", [P, 33344], BF, isOutput=True)

    with tile.TileContext(nc) as tc:
        with (
            tc.tile_pool(name="persist", bufs=1) as persist,
            tc.tile_pool(name="work", bufs=2) as work,
            tc.tile_pool(name="raws", bufs=5) as raws,
            tc.tile_pool(name="work3", bufs=10) as work3,
            tc.tile_pool(name="ps_sc", bufs=2, space="PSUM") as ps_sc_pool,
            tc.tile_pool(name="ps_av", bufs=2, space="PSUM") as ps_av_pool,
            tc.tile_pool(name="ps_mm", bufs=2, space="PSUM") as ps_mm_pool,
        ):
            xT = persist.tile([P, 8, NTOK], BF)
            wqk = persist.tile([P, 8, 2048], BF)
            wv = persist.tile([P, 8, DIM], BF)
            wp = persist.tile([P, 8, DIM], BF)
            cosp = persist.tile([P, NTOK], BF)
            sinp = persist.tile([P, NTOK], BF)
            ptm = persist.tile([P, P], BF)
            bcw = persist.tile([1, 2 * P], BF)
            biasT = persist.tile([P, 8], F32)
            # rope'd QK, merged layout: partition 64*(h%2) + d, chunk = pair
            qm = persist.tile([P, 8, NTOK], BF)
            km = persist.tile([P, 8, NTOK], BF)
            # V with ones column: [k-token partitions, kc, head, 65]
            vaug = persist.tile([P, 8, HEADS * (HD + 1)], BF)
            # attention out, transposed: partition 64*(h%2)+d, chunk h//2
            outT = persist.tile([P, 8, NTOK], BF)

            vaug_r = vaug.rearrange("p n (h e) -> p n h e", e=HD + 1)

            # ---------------- input DMAs (consolidated, priority order)
            # DRAM params are partition-major ([128, 8*ncol], chunks c
            # contiguous per row) so every transfer has >=1KB runs.
            def load3d(dst, src_e, cols, ncol):
                src = src_e.rearrange("p (c n) -> p c n", c=8)
                src = src[:, :, cols] if cols is not None else src[:, :, :]
                nc.sync.dma_start(out=dst, in_=src)

            def load3d_s(dst, src_e, cols, ncol):
                src = src_e.rearrange("p (c n) -> p c n", c=8)
                src = src[:, :, cols] if cols is not None else src[:, :, :]
                nc.scalar.dma_start(out=dst, in_=src)

            load3d(xT[:, :, 0:QT], xT_e, slice(0, QT), NTOK)
            load3d(wqk[:, :, 0:512], wqk_e, slice(0, 512), 2048)
            nc.sync.dma_start(out=cosp[:], in_=cos_e[:, :])
            nc.sync.dma_start(out=sinp[:], in_=sin_e[:, :])
            nc.sync.dma_start(out=ptm[:], in_=pt_e[:, :])
            load3d(xT[:, :, QT:NTOK], xT_e, slice(QT, NTOK), NTOK)
            load3d(wv[:, :, :], wv_e, None, DIM)
            nc.sync.dma_start(out=bcw[:], in_=bcw_e[:, :])
            load3d_s(wqk[:, :, 512:1024], wqk_e, slice(512, 1024), 2048)
            load3d_s(wqk[:, :, 1024:1536], wqk_e, slice(1024, 1536), 2048)
            load3d_s(wqk[:, :, 1536:2048], wqk_e, slice(1536, 2048), 2048)
            nc.scalar.dma_start(out=biasT[:], in_=b_e[:, :])
            load3d_s(wp[:, :, :], wp_e, None, DIM)

            # ---------------- V = x @ Wv^T, natural orientation
            def v_units():
                for tt in range(8):
                    for g in range(2):
                        pt = ps_mm_pool.tile([P, QT], F32, tag="ps_mm")
                        for cc in range(8):
                            nc.tensor.matmul(
                                pt[:],
                                xT[:, cc, tt * P:(tt + 1) * P],
                                wv[:, cc, g * QT:(g + 1) * QT],
                                start=(cc == 0), stop=(cc == 7))
                        nc.scalar.copy(
                            vaug_r[:, tt, g * 8:(g + 1) * 8, 0:HD],
                            pt[:].rearrange("p (h d) -> p h d", d=HD))
                        yield
                    nc.vector.memset(vaug_r[:, tt, :, HD:HD + 1], 1.0)

            # ---------------- QKV^T for one quad (pairs 2m, 2m+1) + RoPE
            def rope_tail(item):
                dst, pp, m, ts_, ca, sb = item
                ptl = ps_mm_pool.tile([P, QT], F32, tag="ps_mm")
                nc.tensor.matmul(ptl[:], ptm[:], sb[:],
                                 start=True, stop=True)
                nc.vector.tensor_add(
                    dst[:, 2 * m + pp, ts_], ca[:], ptl[:])

            def qkv_units(m):
                stash = []
                for t2 in range(NQ):
                    ts_ = slice(t2 * QT, (t2 + 1) * QT)
                    for (dst, pp), gi in (((qm, 0), 0), ((km, 0), 2),
                                          ((qm, 1), 1), ((km, 1), 3)):
                        pr = ps_mm_pool.tile([P, QT], F32, tag="ps_mm")
                        col0 = m * 512 + gi * P
                        for cc in range(8):
                            nc.tensor.matmul(
                                pr[:],
                                wqk[:, cc, col0:col0 + P],
                                xT[:, cc, ts_],
                                start=(cc == 0), stop=(cc == 7))
                        # emit permute+rope of the group 2 slots back first
                        if len(stash) >= 2:
                            rope_tail(stash.pop(0))
                        ca = raws.tile([P, QT], BF, tag="ca")
                        sb = raws.tile([P, QT], BF, tag="sb")
                        nc.vector.tensor_tensor(
                            ca[:], pr[:], cosp[:, ts_], op=AL.mult)
                        nc.vector.tensor_tensor(
                            sb[:], pr[:], sinp[:, ts_], op=AL.mult)
                        stash.append((dst, pp, m, ts_, ca, sb))
                        yield
                for item in stash:
                    rope_tail(item)
                    yield

            # ---------------- attention for one pair (2 heads, concurrent)
            carry = [None]   # deferred attn@V burst crossing attn tiles

            def attn_units(p, qt):
                qs = slice(qt * QT, (qt + 1) * QT)
                h0, h1 = 2 * p, 2 * p + 1
                pending = None   # previous slot's deferred attn@V burst
                po0 = ps_av_pool.tile([HD + 1, QT], F32, tag="ps_o")
                po1 = ps_av_pool.tile([HD + 1, QT], F32, tag="ps_o")

                def make_av(aTs, kcs, finish):
                    def emit():
                        for aT, kc in zip(aTs, kcs):
                            nc.tensor.matmul(
                                po0[:], vaug_r[:, kc, h0, :], aT[:, 0:QT],
                                start=(kc == 0), stop=(kc == 7))
                            nc.tensor.matmul(
                                po1[:], vaug_r[:, kc, h1, :], aT[:, QT:2 * QT],
                                start=(kc == 0), stop=(kc == 7))
                        if finish:
                            den2 = work.tile([1, 2 * QT], BF, tag="den2")
                            nc.vector.tensor_copy(
                                den2[0:1, 0:QT], po0[HD:HD + 1, :])
                            nc.vector.tensor_copy(
                                den2[0:1, QT:2 * QT], po1[HD:HD + 1, :])
                            rbp = ps_mm_pool.tile([P, QT], F32, tag="ps_mm")
                            nc.tensor.matmul(rbp[:], bcw[0:1, 0:P],
                                             den2[0:1, 0:QT],
                                             start=True, stop=False)
                            nc.tensor.matmul(rbp[:], bcw[0:1, P:2 * P],
                                             den2[0:1, QT:2 * QT],
                                             start=False, stop=True)
                            rbs = work.tile([P, QT], F32, tag="rbs")
                            nc.vector.reciprocal_approx_fast(rbs[:], rbp[:])
                            nc.vector.tensor_tensor(
                                outT[0:HD, p, qs],
                                po0[0:HD, :], rbs[0:HD, :], op=AL.mult)
                            nc.vector.tensor_tensor(
                                outT[HD:2 * HD, p, qs],
                                po1[0:HD, :], rbs[HD:2 * HD, :], op=AL.mult)
                    return emit

                for g4 in range(2):
                    aTs = []
                    kcs = list(range(4 * g4, 4 * g4 + 4))
                    for kc in kcs:
                        ks = slice(kc * P, (kc + 1) * P)
                        pss = ps_sc_pool.tile([P, 2 * QT], F32, tag="ps_s")
                        nc.tensor.matmul(
                            pss[:, 0:QT],
                            km[0:HD, p, ks], qm[0:HD, p, qs],
                            start=True, stop=True)
                        nc.tensor.matmul(
                            pss[:, QT:2 * QT],
                            km[HD:2 * HD, p, ks], qm[HD:2 * HD, p, qs],
                            start=True, stop=True)
                        aT = work3.tile([P, 2 * QT], BF, tag="aT")
                        nc.scalar.activation(aT[:], pss[:], AF.Exp,
                                             scale=0.125)
                        aTs.append(aT)
                        if kc == kcs[0] and carry[0] is not None:
                            carry[0]()
                            carry[0] = None
                        if kc % 2 == 1:
                            yield
                    if pending is not None:
                        pending()
                        yield
                    pending = make_av(aTs, kcs, finish=(g4 == 1))
                carry[0] = pending
                yield

            def attn_flush():
                if carry[0] is not None:
                    carry[0]()
                    carry[0] = None
                yield

            # ---------------- output projection + bias
            def proj_units(qt):
                for ot in range(8):
                    os_ = slice(ot * P, (ot + 1) * P)
                    qs = slice(qt * QT, (qt + 1) * QT)
                    pt = ps_mm_pool.tile([P, QT], F32, tag="ps_mm")
                    for cc in range(8):
                        nc.tensor.matmul(
                            pt[:], wp[:, cc, os_], outT[:, cc, qs],
                            start=(cc == 0), stop=(cc == 7))
                    ys = work.tile([P, QT], F32, tag="ys")
                    nc.vector.tensor_scalar_add(ys[:], pt[:], biasT[:, ot:ot + 1])
                    nc.sync.dma_start(out=out_e[os_, qs], in_=ys[:])
                    yield

            def run(gen):
                for _ in gen:
                    pass

            def weave(a, b, ra=3, rb=1):
                """Generator: alternate ra units from a with rb units from b."""
                a, b = iter(a), iter(b)
                alive_a = alive_b = True
                while alive_a or alive_b:
                    for _ in range(ra):
                        if alive_a:
                            try:
                                next(a)
                            except StopIteration:
                                alive_a = False
                            else:
                                yield
                    for _ in range(rb):
                        if alive_b:
                            try:
                                next(b)
                            except StopIteration:
                                alive_b = False
                            else:
                                yield

            def chain(*gens):
                for g in gens:
                    for _ in g:
                        yield

            run(weave(qkv_units(0), v_units(), 2, 3))
            run(weave(chain(attn_units(0, 0), attn_units(1, 0)),
                      qkv_units(1), 3, 2))
            run(weave(chain(attn_units(2, 0), attn_units(3, 0)),
                      qkv_units(2), 3, 2))
            run(weave(chain(attn_units(4, 0), attn_units(5, 0)),
                      qkv_units(3), 3, 2))
            run(chain(attn_units(6, 0), attn_units(7, 0)))
            run(weave(chain(*[attn_units(p, 1) for p in range(8)],
                            attn_flush()),
                      proj_units(0), 7, 1))
            run(proj_units(1))
            if debug:
                nc.sync.dma_start(out=dbg_e[:, 0:8192],
                                  in_=qm.rearrange("p c n -> p (c n)"))
                nc.sync.dma_start(out=dbg_e[:, 8192:16384],
                                  in_=km.rearrange("p c n -> p (c n)"))
                nc.sync.dma_start(out=dbg_e[:, 16384:24704],
                                  in_=vaug.rearrange("p c n -> p (c n)"))
                nc.sync.dma_start(out=dbg_e[:, 24704:24704 + 128],
                                  in_=ptm[:, :])
                nc.sync.dma_start(out=dbg_e[:, 24832:24832 + 128],
                                  in_=cosp[:, 0:128])
                nc.sync.dma_start(out=dbg_e[:, 24960:24960 + 128],
                                  in_=sinp[:, 0:128])
                nc.sync.dma_start(out=dbg_e[:, 25152:33344],
                                  in_=outT.rearrange("p c n -> p (c n)"))

    nc.compile()
    return nc


def _get_nc():
    global _BUILT
    if _BUILT is None:
        _BUILT = _build()
    return _BUILT


# ------------------------------------------------- tracing support (axon)

def _ensure_trace_hooks():
    """Register the NTFF profile hook that the bare agent image's antenv
    stub lacks, and neuter the artifact upload (no bucket in-container)."""
    import types
    import concourse.bass_utils as bu

    bu.upload_artifacts = lambda tmpdir: f"local:{tmpdir}"
    try:
        from antenv.axon_hooks import get_axon_ntff_profile_hook  # noqa: F401
        return
    except ImportError:
        pass
    mod = types.ModuleType("antenv.axon_hooks")
    _state = {"hook": None}
    mod.set_axon_ntff_profile_hook = lambda h: _state.__setitem__("hook", h)
    mod.get_axon_ntff_profile_hook = lambda: _state["hook"]
    import antenv
    sys.modules["antenv.axon_hooks"] = mod
    antenv.axon_hooks = mod
    try:
        from trn_agent_boot.trn_boot import _ntff_profile_via_ctypes
        hook = _ntff_profile_via_ctypes("/opt/axon/libaxon_pjrt.so")
        if hook is not None:
            mod.set_axon_ntff_profile_hook(hook)
    except Exception as e:  # pragma: no cover
        print(f"NTFF hook install failed: {e!r}")


# ----------------------------------------------------------------- kernel()

def kernel(x, Wqkv, Wproj, bproj):
    global LAST_RESULT
    x = np.asarray(x, np.float32)
    Wqkv = np.asarray(Wqkv, np.float32)
    Wproj = np.asarray(Wproj, np.float32)
    bproj = np.asarray(bproj, np.float32)
    B = x.shape[0]

    base = _prep_weights(Wqkv, Wproj, bproj)
    bf = ml_dtypes.bfloat16
    def xprep(xb):
        t = np.ascontiguousarray(xb.T).reshape(8, 128, NTOK)
        return np.ascontiguousarray(
            t.transpose(1, 0, 2).reshape(128, 8 * NTOK)).astype(bf)

    in_maps = [dict(base, xT=xprep(x[b])) for b in range(B)]
    nc = _get_nc()
    trace = bool(os.environ.get("KBENCH_TRACE"))
    if trace:
        _ensure_trace_hooks()
    res = run_bass_kernel_spmd(
        nc, in_maps, core_ids=list(range(B)), trace=trace)
    LAST_RESULT = res
    out = np.stack([np.asarray(res.results[b]["out"]).T for b in range(B)])
    return np.ascontiguousarray(out.astype(np.float32))
